# revision 30
# baseline (speedup 1.0000x reference)
"""CeNN front-end Trainium2 kernel: SBUF-resident fp16 state.

Reference computation (per batch image u [1,H,W]):
    control = conv3x3_same(u, W_B)                         # [64,H,W]
    x0 = control
    x_{k+1} = alpha*x_k + beta*(conv3x3_same(tanh(x_k), WA_eff) + control
                                + bias)
    (WA_eff = W_A with diagonal center taps clamped >= 1), 16 steps.

Distribution: 8 cores = (batch b in 0..3) x (H half). Each core owns a
272-row slab (256 valid + 16 halo rows toward the other half); halo
contamination advances one row per step, so after 16 steps exactly the
16 halo rows are dirty. Zero inter-core communication.

Design (~1.9x over the strip-streaming v1 baseline):
  * The full per-core state X [64ch, 272 rows, 514 cols] lives in SBUF
    as fp16 for all 16 steps - no X DRAM traffic, no halo recompute.
    Layout [128p, 136, 514]: parts 0:64 = ch for slab rows 0..135
    (block A), parts 64:128 = ch for rows 136..271 (block B).
  * C = beta*(control+bias) is precomputed (pass 0) to DRAM in fp16 and
    streamed back once per step; it enters each row's conv PSUM group
    as a 10th identity-weight matmul tap.
  * conv3x3 = 9 fp16 matmul taps (K=64, M=64, N=512). Taps are emitted
    weight-stationary over 4-row blocks: per (block, tap) 8 matmuls
    round-robin over the 4 PE quadrants; a post-trace pass elides the
    per-matmul LDWEIGHTS the compiler emits, keeping 1 per (tap,
    quadrant, block).
  * Consecutive blocks alternate between two 4-bank PSUM sets (psA for
    even rows, psB for odd rows), so a block's matmuls never wait on
    the previous block's drains and the PE stays HAM-warm.
  * Drains are 3 batched DVE scalar_tensor_tensors per block
    (x' = alpha*x + psum): even rows straight [128p], odd rows as two
    64-part ops with cross-partition PSUM reads (quadrant parity puts
    their psum on the opposite partition half).
  * tanh is one batched scalar-engine activation per block.
  * Seam rows 135/136 take their <=3 cross-block taps from per-step
    tanh scratch tiles into the seam blocks' spare PSUM bank, folded
    in with one extra DVE add.
  * Pass 0 computes control with a K=10 float32r im2col matmul (ones
    row for bias + 9 shifted u taps staged with as-strided DMA views,
    3 DMAs per 4-row chunk), evacuating x0 (scalar engine) and C (DVE)
    in parallel. fp32r matmuls may only target PSUM partition 0, so
    both halves' evacuations read psum parts 0:64 cross-partition.
  * Output is written as fp16 and upcast on the host.
"""

import numpy as np
import bass_rust

import concourse.bacc as bacc
import concourse.tile as tile
from concourse import mybir
from concourse.bass_utils import run_bass_kernel_spmd

F32 = mybir.dt.float32
F32R = mybir.dt.float32r
F16 = mybir.dt.float16
AF = mybir.ActivationFunctionType
ALU = mybir.AluOpType

RH = 136          # tile rows per partition half
SLAB = 272
W = 512
WP = 514
UROWS = 274
RC0 = 4           # pass-0 chunk rows

FULL_CFG = dict(NSTEPS=16)


def _blocks():
    """(r0, nrows) tile-row blocks of 4. Seam blocks (first/last) are
    short so a PSUM bank in their set is free for the foreign-tap
    group. Consecutive blocks alternate between two 4-bank PSUM sets,
    so a block's matmuls never wait on the previous block's drains."""
    bl = [(0, 3)]
    r = 3
    while r < RH - 1:
        bl.append((r, 4))
        r += 4
    bl.append((RH - 1, 1))
    return bl


def _elide_ldweights(nc):
    """Drop LDWEIGHTS whose (quadrant, weights-AP) matches the previous
    load on that quadrant: the PE keeps per-quadrant stationary weights,
    so repeated loads are redundant. 16-bit dtypes only (fp32/f32r
    matmuls must self-load). Waits on an elided load move to the
    following instruction."""
    n_del = 0
    for blk in nc.main_func.blocks:
        insts = blk.instructions
        last = {}
        keep = []
        for idx, inst in enumerate(insts):
            if isinstance(inst, mybir.InstLdweights):
                ap = inst.ins[0]
                tp = inst.tile_position
                tp = tuple(tp) if tp is not None else None
                sig = (ap.memref, ap.offset, str(ap.ap), str(ap.dtype))
                elig = (
                    ap.dtype in (mybir.dt.float16, mybir.dt.bfloat16)
                    and tp is not None
                )
                if elig and last.get(tp) == sig:
                    si = inst.sync_info
                    if si is not None and list(si.on_update):
                        keep.append(inst)
                        continue
                    if si is not None and list(si.on_wait):
                        nxt = insts[idx + 1]
                        nsi = nxt.sync_info
                        if nsi is None:
                            nxt.sync_info = mybir.SyncInfo(
                                on_wait=list(si.on_wait), on_update=[])
                        else:
                            nsi.on_wait = list(nsi.on_wait) + list(si.on_wait)
                    n_del += 1
                    continue
                if tp is not None:
                    last[tp] = sig
            keep.append(inst)
        blk.instructions[:] = keep
    return n_del


def build(cfg):
    NSTEPS = cfg["NSTEPS"]
    nc = bacc.Bacc("TRN2", target_bir_lowering=False, debug=False,
                   num_devices=8)

    u_in = nc.dram_tensor("u_in", [UROWS, WP], F32R, kind="ExternalInput")
    wa_in = nc.dram_tensor("wa_in", [64, 10, 64], F16, kind="ExternalInput")
    wb_in = nc.dram_tensor("wb_in", [10, 64], F32R, kind="ExternalInput")
    nbias_in = nc.dram_tensor("nbias_in", [64, 1], F32, kind="ExternalInput")
    alpha_in = nc.dram_tensor("alpha_in", [1, 1], F32, kind="ExternalInput")
    Cd = nc.dram_tensor("Cd", [128, RH, W], F16, kind="Internal")
    x_out = nc.dram_tensor("x_out", [128, RH, W], F16, kind="ExternalOutput")

    def shifted_u(srow0, kh):
        # [3(kw), RC0(r), W(c)] overlapping window view of padded u
        base = u_in[srow0 + kh:srow0 + kh + RC0, 0:W]
        ap = base.copy()
        ap.ap = bass_rust.VecI64Pair([[1, 3], [WP, RC0], [1, W]])
        return ap

    with tile.TileContext(nc) as tc:
        with tc.tile_pool(name="singles", bufs=1) as singles:
            wa_t = singles.tile([128, 10, 64], F16)
            nc.sync.dma_start(out=wa_t[0:64], in_=wa_in[:, :, :])
            nc.sync.dma_start(out=wa_t[64:128], in_=wa_in[:, :, :])
            wb_t = singles.tile([10, 64], F32R)
            nc.sync.dma_start(out=wb_t, in_=wb_in[:, :])
            nbias_t = singles.tile([128, 1], F32)
            nc.sync.dma_start(out=nbias_t[0:64], in_=nbias_in[:, :])
            nc.sync.dma_start(out=nbias_t[64:128], in_=nbias_in[:, :])
            alpha_t = singles.tile([128, 1], F32)
            nc.sync.dma_start(out=alpha_t,
                              in_=alpha_in[:, :].to_broadcast((128, 1)))
            beta_t = singles.tile([128, 1], F32)
            nc.vector.tensor_scalar(out=beta_t, in0=alpha_t, scalar1=-1.0,
                                    scalar2=1.0, op0=ALU.mult, op1=ALU.add)

            xs = singles.tile([128, RH, WP], F16)
            # only the pad columns need zeroing; pass 0 writes cols 1:513
            nc.vector.memset(xs[:, :, 0:1], 0.0)
            nc.vector.memset(xs[:, :, 513:514], 0.0)

            # ---------------- pass 0: control -> x0 (SBUF), C (DRAM) ------
            # K=10 f32r im2col matmul (ones row for bias + 9 shifted u
            # taps). 8-row chunks (3 strided im2col DMAs + 1 C store
            # each) into two 4-bank PSUM tiles, evacuated with batched
            # activations (x0) + batched DVE stts (C).
            with tc.tile_pool(name="u9p", bufs=4) as u9p, \
                 tc.tile_pool(name="csp", bufs=3) as csp, \
                 tc.tile_pool(name="p0ps", bufs=2, space="PSUM") as p0ps:
                for iter_ in range(2 * (RH // RC0)):
                    half, chk = iter_ % 2, iter_ // 2
                    if True:
                        pr = slice(half * 64, half * 64 + 64)
                        trow0 = RC0 * chk
                        srow0 = half * RH + trow0
                        u9 = u9p.tile([10, RC0, W], F32R, tag="u9")
                        nc.vector.memset(u9[0:1].bitcast(F32), 1.0)
                        for kh in range(3):
                            eng = nc.gpsimd if kh < 2 else nc.sync
                            eng.dma_start(
                                out=u9[1 + 3 * kh:4 + 3 * kh],
                                in_=shifted_u(srow0, kh))
                        cst = csp.tile([128, RC0, W], F16, tag="cst")
                        pc = p0ps.tile([64, RC0, W], F32, tag="pc")
                        for r in range(RC0):
                            nc.tensor.matmul(pc[:, r, :], wb_t, u9[:, r, :],
                                             start=True, stop=True,
                                             tile_position=(0, 0),
                                             skip_group_check=True)
                        nc.scalar.activation(
                            out=xs[pr, trow0:trow0 + RC0, 1:513], in_=pc,
                            func=AF.Identity, bias=nbias_t[pr], scale=1.0)
                        nc.vector.scalar_tensor_tensor(
                            out=cst[pr], in0=pc, scalar=beta_t[pr],
                            in1=cst[pr], op0=ALU.mult, op1=ALU.bypass)
                        nc.sync.dma_start(out=Cd[pr, trow0:trow0 + RC0, :],
                                          in_=cst[pr])

            # ---------------- 16 steps, X resident in SBUF ----------------
            with tc.tile_pool(name="thp", bufs=6) as thp, \
                 tc.tile_pool(name="seamp", bufs=2) as seamp, \
                 tc.tile_pool(name="ctp", bufs=8) as ctp, \
                 tc.tile_pool(name="psp", bufs=2, space="PSUM") as psp:
                blocks = _blocks()
                for k in range(NSTEPS):
                    seam_t = seamp.tile([128, WP], F16, tag="seam")
                    nc.scalar.activation(out=seam_t[0:64],
                                         in_=xs[0:64, RH - 1, :], func=AF.Tanh)
                    nc.scalar.activation(out=seam_t[64:128],
                                         in_=xs[64:128, 0, :], func=AF.Tanh)
                    th = {}
                    for (r0, blk) in blocks:
                        ct = ctp.tile([128, blk, W], F16, tag="ct")
                        nc.gpsimd.dma_start(out=ct, in_=Cd[:, r0:r0 + blk, :])
                        lo = 0 if r0 == 0 else r0 + 1
                        hi = min(r0 + blk + 1, RH)
                        if hi > lo:
                            tt = thp.tile([128, 5, WP], F16, tag="tt")
                            nc.scalar.activation(out=tt[:, 0:hi - lo, :],
                                                 in_=xs[:, lo:hi, :],
                                                 func=AF.Tanh)
                            for r in range(lo, hi):
                                th[r] = (tt, r - lo)
                        psA = psp.tile([128, 2, W], F32, name="psA", tag="psA")
                        psB = psp.tile([128, 2, W], F32, name="psB", tag="psB")
                        nA = len(range(r0, r0 + blk, 2))
                        nB = len(range(r0 + 1, r0 + blk, 2))
                        pf = None
                        if r0 == 0:
                            pf = psB[:, 1, :]
                        elif r0 == RH - 1:
                            pf = psA[:, 1, :]
                        st = [[True, True] for _ in range(blk)]
                        for t in range(10):
                            for jj in range(blk):
                                r = r0 + jj
                                phA = jj & 1
                                bank = (psA[:, jj // 2, :] if phA == 0
                                        else psB[:, jj // 2, :])
                                for side in range(2):
                                    ph = phA if side == 0 else 1 - phA
                                    pr = slice(side * 64, side * 64 + 64)
                                    out = bank[ph * 64:ph * 64 + 64, :]
                                    tp = (side * 64, ph * 64)
                                    if t == 9:
                                        rhs = ct[pr, jj, :]
                                    else:
                                        kh, kw = divmod(t, 3)
                                        sr = r + kh - 1
                                        if side == 0 and sr < 0:
                                            continue
                                        if side == 1 and sr > RH - 1:
                                            continue
                                        if side == 0 and sr > RH - 1:
                                            # A row 135 <- tanh(B row 0)
                                            nc.tensor.matmul(
                                                pf[0:64],
                                                wa_t[64:128, t, :],
                                                seam_t[64:128, kw:kw + W],
                                                start=(t == 6), stop=(t == 8),
                                                tile_position=(64, 0),
                                                skip_group_check=True)
                                            continue
                                        if side == 1 and sr < 0:
                                            # B row 0 <- tanh(A row 135)
                                            nc.tensor.matmul(
                                                pf[64:128],
                                                wa_t[0:64, t, :],
                                                seam_t[0:64, kw:kw + W],
                                                start=(t == 0), stop=(t == 2),
                                                tile_position=(0, 64),
                                                skip_group_check=True)
                                            continue
                                        tsr, slot = th[sr]
                                        rhs = tsr[pr, slot, kw:kw + W]
                                    nc.tensor.matmul(
                                        out, wa_t[pr, t, :], rhs,
                                        start=st[jj][side], stop=(t == 9),
                                        tile_position=tp,
                                        skip_group_check=True)
                                    st[jj][side] = False
                        # consolidated drains: even rows from psA (straight),
                        # odd rows from psB (partition-crossed)
                        nc.vector.scalar_tensor_tensor(
                            out=xs[:, r0:r0 + blk:2, 1:513],
                            in0=xs[:, r0:r0 + blk:2, 1:513],
                            scalar=alpha_t, in1=psA[:, 0:nA, :],
                            op0=ALU.mult, op1=ALU.add)
                        if nB:
                            nc.vector.scalar_tensor_tensor(
                                out=xs[0:64, r0 + 1:r0 + blk:2, 1:513],
                                in0=xs[0:64, r0 + 1:r0 + blk:2, 1:513],
                                scalar=alpha_t[0:64], in1=psB[64:128, 0:nB, :],
                                op0=ALU.mult, op1=ALU.add)
                            nc.vector.scalar_tensor_tensor(
                                out=xs[64:128, r0 + 1:r0 + blk:2, 1:513],
                                in0=xs[64:128, r0 + 1:r0 + blk:2, 1:513],
                                scalar=alpha_t[64:128], in1=psB[0:64, 0:nB, :],
                                op0=ALU.mult, op1=ALU.add)
                        if r0 == 0:
                            nc.vector.scalar_tensor_tensor(
                                out=xs[64:128, 0, 1:513],
                                in0=xs[64:128, 0, 1:513],
                                scalar=1.0, in1=psB[64:128, 1, :],
                                op0=ALU.bypass, op1=ALU.add)
                        if r0 == RH - 1:
                            nc.vector.scalar_tensor_tensor(
                                out=xs[0:64, RH - 1, 1:513],
                                in0=xs[0:64, RH - 1, 1:513],
                                scalar=1.0, in1=psA[0:64, 1, :],
                                op0=ALU.bypass, op1=ALU.add)


            for oc in range(4):
                r0o, r1o = 34 * oc, 34 * (oc + 1)
                nc.sync.dma_start(out=x_out[0:64, r0o:r1o, :],
                                  in_=xs[0:64, r0o:r1o, 1:513])
                nc.sync.dma_start(out=x_out[64:128, r0o:r1o, :],
                                  in_=xs[64:128, r0o:r1o, 1:513])

    n_del = _elide_ldweights(nc)
    build.last_elided = n_del
    nc.compile()
    return nc


def host_prep(u, W_B, W_A, bias, alpha_logit, cfg):
    u = np.asarray(u, dtype=np.float32)
    B, _, H, Wc = u.shape

    alpha = np.float32(1.0 / (1.0 + np.exp(-np.float64(alpha_logit))))
    beta = np.float32(1.0) - alpha

    WAe = np.array(W_A, dtype=np.float32).copy()
    idx = np.arange(64)
    WAe[idx, idx, 1, 1] = np.maximum(WAe[idx, idx, 1, 1], np.float32(1.0))

    wa = np.zeros((64, 10, 64), dtype=np.float32)
    for t in range(9):
        kh, kw = divmod(t, 3)
        wa[:, t, :] = (beta * WAe[:, :, kh, kw]).T     # [cin, cout]
    wa[:, 9, :] = np.eye(64, dtype=np.float32)
    wa = wa.astype(np.float16)

    bias_vec = np.array(bias, dtype=np.float32).reshape(64)
    wb = np.zeros((10, 64), dtype=np.float32)
    wb[0, :] = bias_vec
    for t in range(9):
        kh, kw = divmod(t, 3)
        wb[1 + t, :] = W_B[:, 0, kh, kw]
    nbias = (-bias_vec).reshape(64, 1).astype(np.float32)
    alpha_arr = np.full((1, 1), alpha, dtype=np.float32)

    in_maps = []
    for core in range(8):
        b, h = divmod(core, 2)
        img = u[b, 0]                                   # [H, 512]
        us = np.zeros((UROWS, WP), dtype=np.float32)
        if h == 0:
            us[1:UROWS, 1:513] = img[0:SLAB + 1]
        else:
            us[0:UROWS - 1, 1:513] = img[H - SLAB - 1:H]
        in_maps.append({
            "u_in": us,
            "wa_in": wa,
            "wb_in": wb,
            "nbias_in": nbias,
            "alpha_in": alpha_arr,
        })
    return in_maps


_NC_CACHE = {}


def _get_nc():
    if "nc" not in _NC_CACHE:
        _NC_CACHE["nc"] = build(FULL_CFG)
    return _NC_CACHE["nc"]


def kernel(u, W_B, W_A, bias, alpha_logit, _trace=False):
    u = np.asarray(u, dtype=np.float32)
    B, _, H, Wc = u.shape
    nc = _get_nc()
    in_maps = host_prep(u, W_B, W_A, bias, alpha_logit, FULL_CFG)
    res = run_bass_kernel_spmd(nc, in_maps, core_ids=list(range(8)),
                               trace=_trace)
    VALID = H // 2                                      # 256
    out = np.zeros((B, 64, H, Wc), dtype=np.float32)
    for core in range(8):
        b, h = divmod(core, 2)
        xo = np.asarray(res.results[core]["x_out"], dtype=np.float32)
        slab = np.concatenate([xo[0:64], xo[64:128]], axis=1)   # [64,272,512]
        if h == 0:
            out[b, :, 0:VALID, :] = slab[:, 0:VALID, :]
        else:
            out[b, :, VALID:H, :] = slab[:, SLAB - VALID:SLAB, :]
    kernel._last_results = res
    return out


# revision 31
# speedup vs baseline: 1.0660x; 1.0660x over previous
"""CeNN front-end Trainium2 kernel: SBUF-resident fp16 state.

Reference computation (per batch image u [1,H,W]):
    control = conv3x3_same(u, W_B)                         # [64,H,W]
    x0 = control
    x_{k+1} = alpha*x_k + beta*(conv3x3_same(tanh(x_k), WA_eff) + control
                                + bias)
    (WA_eff = W_A with diagonal center taps clamped >= 1), 16 steps.

Distribution: 8 cores = (batch b in 0..3) x (H half). Each core owns a
272-row slab (256 valid + 16 halo rows toward the other half); halo
contamination advances one row per step, so after 16 steps exactly the
16 halo rows are dirty. Zero inter-core communication.

Design (~1.9x over the strip-streaming v1 baseline):
  * The full per-core state X [64ch, 272 rows, 514 cols] lives in SBUF
    as fp16 for all 16 steps - no X DRAM traffic, no halo recompute.
    Layout [128p, 136, 514]: parts 0:64 = ch for slab rows 0..135
    (block A), parts 64:128 = ch for rows 136..271 (block B).
  * C = beta*(control+bias) is precomputed (pass 0) to DRAM in fp16 and
    streamed back once per step; it enters each row's conv PSUM group
    as a 10th identity-weight matmul tap.
  * conv3x3 = 9 fp16 matmul taps (K=64, M=64, N=512). Taps are emitted
    weight-stationary over 4-row blocks: per (block, tap) 8 matmuls
    round-robin over the 4 PE quadrants; a post-trace pass elides the
    per-matmul LDWEIGHTS the compiler emits, keeping 1 per (tap,
    quadrant, block).
  * Consecutive blocks alternate between two 4-bank PSUM sets (psA for
    even rows, psB for odd rows), so a block's matmuls never wait on
    the previous block's drains and the PE stays HAM-warm.
  * Drains are 3 batched DVE scalar_tensor_tensors per block
    (x' = alpha*x + psum): even rows straight [128p], odd rows as two
    64-part ops with cross-partition PSUM reads (quadrant parity puts
    their psum on the opposite partition half).
  * tanh is one batched scalar-engine activation per block.
  * Seam rows 135/136 take their <=3 cross-block taps from per-step
    tanh scratch tiles into the seam blocks' spare PSUM bank, folded
    in with one extra DVE add.
  * Pass 0 computes control with a K=10 float32r im2col matmul (ones
    row for bias + 9 shifted u taps staged with as-strided DMA views,
    3 DMAs per 4-row chunk), evacuating x0 (scalar engine) and C (DVE)
    in parallel. fp32r matmuls may only target PSUM partition 0, so
    both halves' evacuations read psum parts 0:64 cross-partition.
  * Output is written as fp16 and upcast on the host.
"""

import numpy as np
import bass_rust

import concourse.bacc as bacc
import concourse.tile as tile
from concourse import mybir
from concourse.bass_utils import run_bass_kernel_spmd

F32 = mybir.dt.float32
F32R = mybir.dt.float32r
F16 = mybir.dt.float16
AF = mybir.ActivationFunctionType
ALU = mybir.AluOpType

RH = 136          # tile rows per partition half
SLAB = 272
W = 512
WP = 514
UROWS = 274
RC0 = 4           # pass-0 chunk rows

FULL_CFG = dict(NSTEPS=16)


def _blocks():
    """(r0, nrows) tile-row blocks of 4. Seam blocks (first/last) are
    short so a PSUM bank in their set is free for the foreign-tap
    group. Consecutive blocks alternate between two 4-bank PSUM sets,
    so a block's matmuls never wait on the previous block's drains."""
    bl = [(0, 3)]
    r = 3
    while r < RH - 1:
        bl.append((r, 4))
        r += 4
    bl.append((RH - 1, 1))
    return bl


def _elide_ldweights(nc):
    """Drop LDWEIGHTS whose (quadrant, weights-AP) matches the previous
    load on that quadrant: the PE keeps per-quadrant stationary weights,
    so repeated loads are redundant. 16-bit dtypes only (fp32/f32r
    matmuls must self-load). Waits on an elided load move to the
    following instruction."""
    n_del = 0
    for blk in nc.main_func.blocks:
        insts = blk.instructions
        last = {}
        keep = []
        for idx, inst in enumerate(insts):
            if isinstance(inst, mybir.InstLdweights):
                ap = inst.ins[0]
                tp = inst.tile_position
                tp = tuple(tp) if tp is not None else None
                sig = (ap.memref, ap.offset, str(ap.ap), str(ap.dtype))
                elig = (
                    ap.dtype in (mybir.dt.float16, mybir.dt.bfloat16)
                    and tp is not None
                )
                if elig and last.get(tp) == sig:
                    si = inst.sync_info
                    if si is not None and list(si.on_update):
                        keep.append(inst)
                        continue
                    if si is not None and list(si.on_wait):
                        nxt = insts[idx + 1]
                        nsi = nxt.sync_info
                        if nsi is None:
                            nxt.sync_info = mybir.SyncInfo(
                                on_wait=list(si.on_wait), on_update=[])
                        else:
                            nsi.on_wait = list(nsi.on_wait) + list(si.on_wait)
                    n_del += 1
                    continue
                if tp is not None:
                    last[tp] = sig
            keep.append(inst)
        blk.instructions[:] = keep
    return n_del


def build(cfg):
    NSTEPS = cfg["NSTEPS"]
    nc = bacc.Bacc("TRN2", target_bir_lowering=False, debug=False,
                   num_devices=8)

    u_in = nc.dram_tensor("u_in", [UROWS, WP], F32R, kind="ExternalInput")
    wa_in = nc.dram_tensor("wa_in", [64, 10, 64], F16, kind="ExternalInput")
    wb_in = nc.dram_tensor("wb_in", [10, 64], F32R, kind="ExternalInput")
    nbias_in = nc.dram_tensor("nbias_in", [64, 1], F32, kind="ExternalInput")
    alpha_in = nc.dram_tensor("alpha_in", [1, 1], F32, kind="ExternalInput")
    Cd = nc.dram_tensor("Cd", [128, RH, W], F16, kind="Internal")
    x_out = nc.dram_tensor("x_out", [128, RH, W], F16, kind="ExternalOutput")

    def shifted_u(srow0, kh):
        # [3(kw), RC0(r), W(c)] overlapping window view of padded u
        base = u_in[srow0 + kh:srow0 + kh + RC0, 0:W]
        ap = base.copy()
        ap.ap = bass_rust.VecI64Pair([[1, 3], [WP, RC0], [1, W]])
        return ap

    with tile.TileContext(nc) as tc:
        with tc.tile_pool(name="singles", bufs=1) as singles:
            wa_t = singles.tile([128, 10, 64], F16)
            nc.sync.dma_start(out=wa_t[0:64], in_=wa_in[:, :, :])
            nc.sync.dma_start(out=wa_t[64:128], in_=wa_in[:, :, :])
            wb_t = singles.tile([10, 64], F32R)
            nc.sync.dma_start(out=wb_t, in_=wb_in[:, :])
            nbias_t = singles.tile([128, 1], F32)
            nc.sync.dma_start(out=nbias_t[0:64], in_=nbias_in[:, :])
            nc.sync.dma_start(out=nbias_t[64:128], in_=nbias_in[:, :])
            alpha_t = singles.tile([128, 1], F32)
            nc.sync.dma_start(out=alpha_t,
                              in_=alpha_in[:, :].to_broadcast((128, 1)))
            beta_t = singles.tile([128, 1], F32)
            nc.vector.tensor_scalar(out=beta_t, in0=alpha_t, scalar1=-1.0,
                                    scalar2=1.0, op0=ALU.mult, op1=ALU.add)

            xs = singles.tile([128, RH, WP], F16)
            # only the pad columns need zeroing; pass 0 writes cols 1:513
            nc.vector.memset(xs[:, :, 0:1], 0.0)
            nc.vector.memset(xs[:, :, 513:514], 0.0)

            # ---------------- pass 0: control -> x0 (SBUF), C (DRAM) ------
            # K=10 f32r im2col matmul (ones row for bias + 9 shifted u
            # taps). 8-row chunks (3 strided im2col DMAs + 1 C store
            # each) into two 4-bank PSUM tiles, evacuated with batched
            # activations (x0) + batched DVE stts (C).
            with tc.tile_pool(name="u9p", bufs=4) as u9p, \
                 tc.tile_pool(name="csp", bufs=3) as csp, \
                 tc.tile_pool(name="p0ps", bufs=2, space="PSUM") as p0ps:
                for iter_ in range(2 * (RH // RC0)):
                    half, chk = iter_ % 2, iter_ // 2
                    if True:
                        pr = slice(half * 64, half * 64 + 64)
                        trow0 = RC0 * chk
                        srow0 = half * RH + trow0
                        u9 = u9p.tile([10, RC0, W], F32R, tag="u9")
                        nc.vector.memset(u9[0:1].bitcast(F32), 1.0)
                        for kh in range(3):
                            nc.gpsimd.dma_start(
                                out=u9[1 + 3 * kh:4 + 3 * kh],
                                in_=shifted_u(srow0, kh))
                        cst = csp.tile([128, RC0, W], F16, tag="cst")
                        pc = p0ps.tile([64, RC0, W], F32, tag="pc")
                        for r in range(RC0):
                            nc.tensor.matmul(pc[:, r, :], wb_t, u9[:, r, :],
                                             start=True, stop=True,
                                             tile_position=(0, 0),
                                             skip_group_check=True)
                        nc.scalar.activation(
                            out=xs[pr, trow0:trow0 + RC0, 1:513], in_=pc,
                            func=AF.Identity, bias=nbias_t[pr], scale=1.0)
                        nc.vector.scalar_tensor_tensor(
                            out=cst[pr], in0=pc, scalar=beta_t[pr],
                            in1=cst[pr], op0=ALU.mult, op1=ALU.bypass)
                        nc.sync.dma_start(out=Cd[pr, trow0:trow0 + RC0, :],
                                          in_=cst[pr])

            # ---------------- 16 steps, X resident in SBUF ----------------
            with tc.tile_pool(name="thp", bufs=6) as thp, \
                 tc.tile_pool(name="seamp", bufs=2) as seamp, \
                 tc.tile_pool(name="ctp", bufs=8) as ctp, \
                 tc.tile_pool(name="psp", bufs=2, space="PSUM") as psp:
                blocks = _blocks()
                for k in range(NSTEPS):
                    seam_t = seamp.tile([128, WP], F16, tag="seam")
                    nc.scalar.activation(out=seam_t[0:64],
                                         in_=xs[0:64, RH - 1, :], func=AF.Tanh)
                    nc.scalar.activation(out=seam_t[64:128],
                                         in_=xs[64:128, 0, :], func=AF.Tanh)
                    th = {}
                    for (r0, blk) in blocks:
                        ct = ctp.tile([128, blk, W], F16, tag="ct")
                        nc.gpsimd.dma_start(out=ct, in_=Cd[:, r0:r0 + blk, :])
                        lo = 0 if r0 == 0 else r0 + 1
                        hi = min(r0 + blk + 1, RH)
                        if hi > lo:
                            tt = thp.tile([128, 5, WP], F16, tag="tt")
                            nc.scalar.activation(out=tt[:, 0:hi - lo, :],
                                                 in_=xs[:, lo:hi, :],
                                                 func=AF.Tanh)
                            for r in range(lo, hi):
                                th[r] = (tt, r - lo)
                        psA = psp.tile([128, 2, W], F32, name="psA", tag="psA")
                        psB = psp.tile([128, 2, W], F32, name="psB", tag="psB")
                        nA = len(range(r0, r0 + blk, 2))
                        nB = len(range(r0 + 1, r0 + blk, 2))
                        pf = None
                        if r0 == 0:
                            pf = psB[:, 1, :]
                        elif r0 == RH - 1:
                            pf = psA[:, 1, :]
                        st = [[True, True] for _ in range(blk)]
                        for t in range(10):
                            for jj in range(blk):
                                r = r0 + jj
                                phA = jj & 1
                                bank = (psA[:, jj // 2, :] if phA == 0
                                        else psB[:, jj // 2, :])
                                for side in range(2):
                                    ph = phA if side == 0 else 1 - phA
                                    pr = slice(side * 64, side * 64 + 64)
                                    out = bank[ph * 64:ph * 64 + 64, :]
                                    tp = (side * 64, ph * 64)
                                    if t == 9:
                                        rhs = ct[pr, jj, :]
                                    else:
                                        kh, kw = divmod(t, 3)
                                        sr = r + kh - 1
                                        if side == 0 and sr < 0:
                                            continue
                                        if side == 1 and sr > RH - 1:
                                            continue
                                        if side == 0 and sr > RH - 1:
                                            # A row 135 <- tanh(B row 0)
                                            nc.tensor.matmul(
                                                pf[0:64],
                                                wa_t[64:128, t, :],
                                                seam_t[64:128, kw:kw + W],
                                                start=(t == 6), stop=(t == 8),
                                                tile_position=(64, 0),
                                                skip_group_check=True)
                                            continue
                                        if side == 1 and sr < 0:
                                            # B row 0 <- tanh(A row 135)
                                            nc.tensor.matmul(
                                                pf[64:128],
                                                wa_t[0:64, t, :],
                                                seam_t[0:64, kw:kw + W],
                                                start=(t == 0), stop=(t == 2),
                                                tile_position=(0, 64),
                                                skip_group_check=True)
                                            continue
                                        tsr, slot = th[sr]
                                        rhs = tsr[pr, slot, kw:kw + W]
                                    nc.tensor.matmul(
                                        out, wa_t[pr, t, :], rhs,
                                        start=st[jj][side], stop=(t == 9),
                                        tile_position=tp,
                                        skip_group_check=True)
                                    st[jj][side] = False
                        # consolidated drains: even rows from psA (straight),
                        # odd rows from psB (partition-crossed)
                        nc.vector.scalar_tensor_tensor(
                            out=xs[:, r0:r0 + blk:2, 1:513],
                            in0=xs[:, r0:r0 + blk:2, 1:513],
                            scalar=alpha_t, in1=psA[:, 0:nA, :],
                            op0=ALU.mult, op1=ALU.add)
                        if nB:
                            nc.vector.scalar_tensor_tensor(
                                out=xs[0:64, r0 + 1:r0 + blk:2, 1:513],
                                in0=xs[0:64, r0 + 1:r0 + blk:2, 1:513],
                                scalar=alpha_t[0:64], in1=psB[64:128, 0:nB, :],
                                op0=ALU.mult, op1=ALU.add)
                            nc.vector.scalar_tensor_tensor(
                                out=xs[64:128, r0 + 1:r0 + blk:2, 1:513],
                                in0=xs[64:128, r0 + 1:r0 + blk:2, 1:513],
                                scalar=alpha_t[64:128], in1=psB[0:64, 0:nB, :],
                                op0=ALU.mult, op1=ALU.add)
                        if r0 == 0:
                            nc.vector.scalar_tensor_tensor(
                                out=xs[64:128, 0, 1:513],
                                in0=xs[64:128, 0, 1:513],
                                scalar=1.0, in1=psB[64:128, 1, :],
                                op0=ALU.bypass, op1=ALU.add)
                        if r0 == RH - 1:
                            nc.vector.scalar_tensor_tensor(
                                out=xs[0:64, RH - 1, 1:513],
                                in0=xs[0:64, RH - 1, 1:513],
                                scalar=1.0, in1=psA[0:64, 1, :],
                                op0=ALU.bypass, op1=ALU.add)


            for oc in range(4):
                r0o, r1o = 34 * oc, 34 * (oc + 1)
                nc.sync.dma_start(out=x_out[0:64, r0o:r1o, :],
                                  in_=xs[0:64, r0o:r1o, 1:513])
                nc.sync.dma_start(out=x_out[64:128, r0o:r1o, :],
                                  in_=xs[64:128, r0o:r1o, 1:513])

    n_del = _elide_ldweights(nc)
    build.last_elided = n_del
    nc.compile()
    return nc


def host_prep(u, W_B, W_A, bias, alpha_logit, cfg):
    u = np.asarray(u, dtype=np.float32)
    B, _, H, Wc = u.shape

    alpha = np.float32(1.0 / (1.0 + np.exp(-np.float64(alpha_logit))))
    beta = np.float32(1.0) - alpha

    WAe = np.array(W_A, dtype=np.float32).copy()
    idx = np.arange(64)
    WAe[idx, idx, 1, 1] = np.maximum(WAe[idx, idx, 1, 1], np.float32(1.0))

    wa = np.zeros((64, 10, 64), dtype=np.float32)
    for t in range(9):
        kh, kw = divmod(t, 3)
        wa[:, t, :] = (beta * WAe[:, :, kh, kw]).T     # [cin, cout]
    wa[:, 9, :] = np.eye(64, dtype=np.float32)
    wa = wa.astype(np.float16)

    bias_vec = np.array(bias, dtype=np.float32).reshape(64)
    wb = np.zeros((10, 64), dtype=np.float32)
    wb[0, :] = bias_vec
    for t in range(9):
        kh, kw = divmod(t, 3)
        wb[1 + t, :] = W_B[:, 0, kh, kw]
    nbias = (-bias_vec).reshape(64, 1).astype(np.float32)
    alpha_arr = np.full((1, 1), alpha, dtype=np.float32)

    in_maps = []
    for core in range(8):
        b, h = divmod(core, 2)
        img = u[b, 0]                                   # [H, 512]
        us = np.zeros((UROWS, WP), dtype=np.float32)
        if h == 0:
            us[1:UROWS, 1:513] = img[0:SLAB + 1]
        else:
            us[0:UROWS - 1, 1:513] = img[H - SLAB - 1:H]
        in_maps.append({
            "u_in": us,
            "wa_in": wa,
            "wb_in": wb,
            "nbias_in": nbias,
            "alpha_in": alpha_arr,
        })
    return in_maps


_NC_CACHE = {}


def _get_nc():
    if "nc" not in _NC_CACHE:
        _NC_CACHE["nc"] = build(FULL_CFG)
    return _NC_CACHE["nc"]


def kernel(u, W_B, W_A, bias, alpha_logit, _trace=False):
    u = np.asarray(u, dtype=np.float32)
    B, _, H, Wc = u.shape
    nc = _get_nc()
    in_maps = host_prep(u, W_B, W_A, bias, alpha_logit, FULL_CFG)
    res = run_bass_kernel_spmd(nc, in_maps, core_ids=list(range(8)),
                               trace=_trace)
    VALID = H // 2                                      # 256
    out = np.zeros((B, 64, H, Wc), dtype=np.float32)
    for core in range(8):
        b, h = divmod(core, 2)
        xo = np.asarray(res.results[core]["x_out"], dtype=np.float32)
        slab = np.concatenate([xo[0:64], xo[64:128]], axis=1)   # [64,272,512]
        if h == 0:
            out[b, :, 0:VALID, :] = slab[:, 0:VALID, :]
        else:
            out[b, :, VALID:H, :] = slab[:, SLAB - VALID:SLAB, :]
    kernel._last_results = res
    return out
